# revision 118
# baseline (speedup 1.0000x reference)
"""DeformConv2d (B=8, C=64, H=W=64, K=3) on 8 Trainium2 NeuronCores.

Batch-parallel: one image per core, all cores run the same Bass/Tile
program.

Math (tent formulation of bilinear sampling; offsets satisfy |dy|,|dx|<1
so each axis' bilinear weight is the 3-point tent (relu(-d), 1-|d|,
relu(d)) on the 3 integer neighbours; out-of-image taps vanish because we
sample a zero-padded image):

  out[o,p] = sum_{k,u,v} wy_u[k,p]*wx_v[k,p]*xpad[c, p+shift(k,u,v)]
             contracted with d_w[o,c,k] over (c,k).

Kernel phases (fp16 data path, fp32 PSUM accumulation):
  0. host-prepadded image pair xpad2 [128, 68*68] fp16 loads in four
     staged chunks (a small lead chunk lets the offset conv start ~3us
     in; rows 64-127 hold the image shifted down one row so a single AP
     covers two row-adjacent terms); small DMAs load the offset-conv
     weights, biases, per-pass d_w and the PE-broadcast selectors.
     Dep-free dummy matmuls warm the PE p-state while the loads land.
  1. offset conv: 6 pair-packed K=128 fp16 matmuls per 512-pixel chunk
     into PSUM, ACT-evacuated (+p_b) to off [18, 4096] fp16 with channels
     host-permuted to dy_0..8 | dx_0..8.
  2. tent weight fields, split by pixel HALF to match the phase-3 sweeps
     (half h = 2048 px, fold row k*4+b'', 4 blocks of 512): refold DMA ->
     8 DVE tent-factor ops -> 9 products (6 DVE + 3 Pool) -> store to
     wtab_h[h] [81, 2048] DRAM (row uv*9+k).  Half 0 needs only conv
     chunks 0-3, so the sweep-0 broadcast stream starts ~20us in; half-1
     work is emitted lazily between sweep-0 passes (where DVE has slack).
  3. TWO SWEEPS of 2048 pixels (PSUM holds the [64,2048] fp32 accumulator
     in banks 0-3 and a [128,2048] fp32 PE-broadcast landing tile in
     banks 4-7).  41 passes per sweep (81 terms pair two-per-pass):
       - 32 DMA passes: ONE DMA broadcasts both terms' weight fields
         (stride-0 middle dim replicates each wtab row across a 64-
         partition half of wexp; descending row pairs fall back to two
         DMAs since negative partition steps are illegal);
       - 9 PE passes: 4 chunked K=2 selector matmuls (sel[2,128] x
         wrows[2,512]) replicate both rows into PSUM, an ACT copy
         evacuates to fp16 SBUF.  This offloads ~9/41 of the 42MB
         broadcast traffic from the DMA ring (the critical resource,
         ~360 B/ns serialized) onto the PE/ACT engines;
       - modulate splits by pixel rows across DVE (2-byte 2x tensor_mul,
         first DR2=26 of 32 rows) and Pool (rest, 0.42 eff);
       - 4 fp16 matmuls accumulate d_w^T @ modulated into PSUM
         (contraction 128 = two terms at once; output <= 512 fp32 = one
         PSUM bank per matmul).
     Per sweep: the final pass stops chunks 2,3 first; two WIDE evacs
     (+d_b, one [64,1024] per store half on DVE and ACT in parallel)
     let both output stores launch ~2us earlier than per-chunk evacs.
     Deep tile pools (13 wexp / 7 mod buffers) hide the ~2.3us DMA
     issue+semaphore latency per broadcast.

kernel(**inputs) takes full (unsharded) inputs, returns the full output.
TimelineSim: 132261 ns/core (baseline: 156595).
"""

import sys

sys.path.insert(0, "/opt/trn_rl_repo")

import numpy as np
import concourse.bass as bass
import concourse.bacc as bacc
import concourse.mybir as mybir
from concourse.tile import TileContext
from concourse.bass_utils import run_bass_kernel_spmd

dt = mybir.dt
AF = mybir.ActivationFunctionType
OP = mybir.AluOpType

B, CIN, H, W = 8, 64, 64, 64
COUT, K = 64, 3
K2 = K * K
HP = H + 4          # 68: 2-pad each side (tent reach is rows/cols -2..65)
FP = HP * HP        # 4624
NPIX = H * W        # 4096
SW = 2048           # sweep width (pixels per phase-3 sweep)
SROWS = SW // W     # 32 image rows per sweep
CH = 512            # accumulation chunk = one PSUM bank of fp32
NCH = SW // CH      # chunks per sweep (4)
FQ = 1024           # fold quarter (pixels per fold block)
FH = 512            # per-half fold block (4 blocks x 512 px per sweep half)
DR2 = 26            # modulate rows on DVE per sweep (of SROWS); rest Pool


def _make_pass_plan():
    """81 (k,u,v) terms -> 41 passes of <=2 terms.

    A pass holds terms (ta, sa) on partitions 0-63 and (tb, sb) on
    64-127.  'split' False means sb == sa+(1,0): one image AP covers both
    halves via the row-shifted copy in xpad2.  'split' True pairs two
    arbitrary-shift terms: two half-modulates (half2's base needs
    sb[0] >= -1 because the shifted copy only pads one row on top)."""
    by_shift = {}
    for k in range(K2):
        kh, kw = k // 3, k % 3
        for u in (-1, 0, 1):
            for v in (-1, 0, 1):
                by_shift.setdefault((kh - 1 + u, kw - 1 + v), []).append((k, u, v))
    passes = []
    leftovers = []
    for sx in range(-2, 3):
        col = {sy: list(by_shift.get((sy, sx), [])) for sy in range(-2, 3)}
        for sy in range(-2, 2):
            a, bb = col[sy], col[sy + 1]
            while a and bb:
                passes.append((a.pop(), (sy, sx), bb.pop(), (sy + 1, sx), False))
        for sy in range(-2, 3):
            while col[sy]:
                leftovers.append((col[sy].pop(), (sy, sx)))
    extras = []
    while len(leftovers) >= 2:
        ta, sa = leftovers.pop()
        tb, sb = leftovers.pop()
        if sb[0] < -1:
            (ta, sa), (tb, sb) = (tb, sb), (ta, sa)
        assert sb[0] >= -1
        extras.append((ta, sa, tb, sb, True))
    if leftovers:
        ta, sa = leftovers.pop()
        extras.append((ta, sa, None, None, True))
    # interleave the DVE-heavier split passes evenly through the schedule
    step = max(1, len(passes) // (len(extras) + 1))
    for i, e in enumerate(extras):
        passes.insert(min(len(passes), (i + 1) * step + i + 4), e)
    n = sum((p[0] is not None) + (p[2] is not None) for p in passes)
    assert n == 81, n
    return passes


PASSES = _make_pass_plan()
NP_ = len(PASSES)   # 41
NCC = NP_ * 64      # fp16 consts columns: dw per pass
NCT = NCC + 256     # + PE-broadcast selectors (normal & flipped) in rows 0-1

# passes whose weight-field broadcast rides the PE instead of the DMA
# ring, spread evenly through the schedule (two-term passes only, past
# the first few so their wtab rows exist by first use).  Symmetric 9/9
# is a sharply-peaked optimum (asymmetric splits all measured worse);
# the per-sweep list structure remains for the compact wrow slots.
N_PE0 = 9
N_PE1 = 9


def _pe_pass_list(n):
    cand = [p for p in range(3, NP_) if PASSES[p][2] is not None]
    if n <= 0:
        return []
    stride = len(cand) / n
    out = []
    for i in range(n):
        c = cand[min(int(i * stride), len(cand) - 1)]
        if c not in out:
            out.append(c)
    return out


PE_SWEEP = [_pe_pass_list(N_PE0), _pe_pass_list(N_PE1)]


def _tid(term):
    k, u, v = term
    return k * 9 + (u + 1) * 3 + (v + 1)


def _wrow(term):
    """Row of the merged wtab [81, 4096] holding this term's field."""
    k, u, v = term
    uv = (u + 1) * 3 + (v + 1)
    return uv * 9 + k


def _pstep(ap):
    return ap.ap[0][0]


def build_nc():
    nc = bacc.Bacc(None, target_bir_lowering=False)
    f32 = dt.float32
    f16 = dt.float16

    xp_d = nc.dram_tensor("xp", [128, FP], f16, kind="ExternalInput")
    pw_d = nc.dram_tensor("pwt", [128, 108], f16, kind="ExternalInput")
    cst_d = nc.dram_tensor("cst", [128, NCT], f16, kind="ExternalInput")
    bia_d = nc.dram_tensor("bia", [128, 2], f32, kind="ExternalInput")
    y_d = nc.dram_tensor("y", [COUT, NPIX], f16, kind="ExternalOutput")
    wtab_h = [nc.dram_tensor(f"wtab{h}", [81, SW], f16, kind="Internal")
              for h in range(2)]

    with TileContext(nc) as tc:
        with (
            tc.tile_pool(name="const", bufs=1) as cp,
            tc.tile_pool(name="wexp", bufs=13) as wp,
            tc.tile_pool(name="mod", bufs=7) as mp,
            tc.tile_pool(name="psout", bufs=1, space="PSUM") as pso,
            tc.tile_pool(name="psw", bufs=1, space="PSUM") as psw,
        ):
            # ---------------- phase 0: loads (one DMA each) ----------------
            xpad2 = cp.tile([128, FP], f16)
            pwt = cp.tile([128, 108], f16)
            nc.scalar.dma_start(out=pwt[:], in_=pw_d[:])
            # staged image load: a small lead chunk lets conv chunk 0 start
            # ~3us earlier; later chunks pipeline under the conv
            XCUTS = [0, 12 * HP, 28 * HP, 44 * HP, FP]
            for xi in range(4):
                nc.sync.dma_start(out=xpad2[:, XCUTS[xi]:XCUTS[xi + 1]],
                                  in_=xp_d[:, XCUTS[xi]:XCUTS[xi + 1]])
            bia = cp.tile([128, 2], f32)
            nc.scalar.dma_start(out=bia[:], in_=bia_d[:])
            cst = cp.tile([128, NCT], f16)
            nc.scalar.dma_start(out=cst[:], in_=cst_d[:])

            pw_sb = pwt[:, :]
            pb_sb = bia[0:2 * K2, 0:1]
            db_sb = bia[0:COUT, 1:2]

            xt = xpad2[:, :]
            xps = _pstep(xt)

            # PE-broadcast selector (host-packed into cst rows 0-1):
            # sel[0, 0:64] = 1 -> partitions 0-63, sel[1, 64:128] = 1.
            sel = cst[0:2, NCC:NCC + 128]
            selflip = cst[0:2, NCC + 128:NCC + 256]

            # ---------------- phase 1: offset conv ----------------
            off_full = cp.tile([128, NPIX], f16)
            off = off_full[0:2 * K2, :]

            # warm the PE p-state while the input DMAs land
            scratch = cp.tile([128, CH], f16)
            nc.vector.memset(scratch[:], 0.0)

            def ps_tile():
                return pso.tile([COUT, SW], f32, name="psacc", tag="psacc")

            cps = ps_tile()
            for w in range(3):
                nc.tensor.matmul(
                    cps[0:2 * K2, 0:CH], scratch[:, 0:2 * K2],
                    scratch[:], start=True, stop=True)
            for c in range(8):
                if c == 4:
                    cps = ps_tile()
                i0 = 8 * c
                pst = cps[0:2 * K2, CH * (c % 4):CH * (c % 4 + 1)]
                for g in range(6):
                    kw = g % 3
                    base = (i0 + (1 if g < 3 else 2)) * HP + kw + 1
                    nc.tensor.matmul(
                        pst,
                        pw_sb[:, 18 * g:18 * g + 18],
                        bass.AP(xt.tensor, xt.offset + base,
                                [[xps, 128], [HP, 8], [1, W]]),
                        start=(g == 0),
                        stop=(g == 5),
                    )
                nc.scalar.activation(off[:, CH * c:CH * (c + 1)], pst,
                                     AF.Identity, bias=pb_sb, scale=1.0)

            # keep the PE busy through phase 2 (tents run on DVE)
            for w in range(18):
                nc.tensor.matmul(
                    cps[0:2 * K2, 0:CH], scratch[:, 0:2 * K2],
                    scratch[:], start=True, stop=True)

            # ---------------- phase 2: tent weight fields ----------------
            # Split by pixel HALF (2048 px = fold blocks b in {2h, 2h+1}),
            # matching the phase-3 sweeps: half-0 products/stores gate
            # sweep 0 and need only conv chunks 0-3, so the broadcast
            # stream starts ~12us earlier.  Half-1 tent work is deferred
            # into the early sweep-0 passes where DVE has slack.
            # Per-half fold layout: row b'*9+k (b-major), b' = b - 2h.
            ot = off[:, :]
            ops_ = _pstep(ot)
            dyf = [cp.tile([4 * K2, FH], f16, name=f"dyf{h}") for h in range(2)]
            dxf = [cp.tile([4 * K2, FH], f16, name=f"dxf{h}") for h in range(2)]
            tent = {n: [cp.tile([4 * K2, FH], f16, name=f"{n}{h}")
                        for h in range(2)] for n in
                    ("ay", "by", "y0", "ax", "bx", "x0")}
            wy = {-1: tent["by"], 0: tent["y0"], 1: tent["ay"]}
            wx = {-1: tent["bx"], 0: tent["x0"], 1: tent["ax"]}

            uv_order = []
            for (ta, sa, tb, sb, split) in PASSES:
                for tt in (ta, tb):
                    if tt is not None and _tid(tt) % 9 not in uv_order:
                        uv_order.append(_tid(tt) % 9)
            for uv in range(9):
                if uv not in uv_order:
                    uv_order.append(uv)

            def emit_refold(h):
                for par, dtile in ((0, dyf[h]), (1, dxf[h])):
                    srcap = bass.AP(ot.tensor,
                                    ot.offset + par * K2 * ops_ + h * SW,
                                    [[ops_, K2], [FH, 4], [1, FH]])
                    nc.sync.dma_start(out=dtile[:], in_=srcap)

            def emit_factor(name, h):
                t = tent[name][h]
                if name == "by":
                    nc.vector.tensor_scalar(t[:], dyf[h][:], -1.0, 0.0,
                                            OP.mult, OP.max)
                elif name == "bx":
                    nc.vector.tensor_scalar(t[:], dxf[h][:], -1.0, 0.0,
                                            OP.mult, OP.max)
                elif name == "ay":
                    nc.vector.tensor_scalar_max(t[:], dyf[h][:], 0.0)
                elif name == "ax":
                    nc.vector.tensor_scalar_max(t[:], dxf[h][:], 0.0)
                elif name == "y0":
                    nc.vector.tensor_add(t[:], tent["ay"][h][:],
                                         tent["by"][h][:])
                    nc.vector.tensor_scalar(t[:], t[:], -1.0, 1.0,
                                            OP.mult, OP.add)
                else:
                    nc.vector.tensor_add(t[:], tent["ax"][h][:],
                                         tent["bx"][h][:])
                    nc.vector.tensor_scalar(t[:], t[:], -1.0, 1.0,
                                            OP.mult, OP.add)

            def emit_product(uv, h, i):
                u, v = uv // 3 - 1, uv % 3 - 1
                wt = mp.tile([4 * K2, FH], f16, name="wprod", tag="wprod",
                             bufs=4)
                peng = nc.gpsimd if i >= 6 else nc.vector
                peng.tensor_mul(wt[:], wy[u][h][:], wx[v][h][:])
                wlt = wt[:, :]
                wps = _pstep(wlt)
                dst = bass.AP(wtab_h[h], uv * K2 * SW,
                              [[SW, K2], [FH, 4], [1, FH]])
                srcw = bass.AP(wlt.tensor, wlt.offset,
                               [[wps, 4 * K2], [1, FH]])
                nc.scalar.dma_start(out=dst, in_=srcw)

            ford = ["by", "bx", "ay", "ax", "y0", "x0"]
            fuv = uv_order[0]
            f1 = {-1: "by", 0: "y0", 1: "ay"}[fuv // 3 - 1]
            f2 = {-1: "bx", 0: "x0", 1: "ax"}[fuv % 3 - 1]
            for f in (f1, f2):
                if f not in ("y0", "x0"):
                    ford.remove(f)
                    ford.insert(0, f)

            def emit_half(h):
                emit_refold(h)
                avail = set()
                done_uv = set()
                for f in ford:
                    emit_factor(f, h)
                    avail.add(f)
                    for i, uv in enumerate(uv_order):
                        u, v = uv // 3 - 1, uv % 3 - 1
                        fy = {-1: "by", 0: "y0", 1: "ay"}[u]
                        fx = {-1: "bx", 0: "x0", 1: "ax"}[v]
                        if uv in done_uv or fy not in avail or fx not in avail:
                            continue
                        done_uv.add(uv)
                        emit_product(uv, h, i)
                assert len(done_uv) == 9

            emit_half(0)
            # half-1 work is emitted lazily inside the sweep-0 pass loop
            deferred = [lambda: emit_refold(1)]
            davail = set()
            ddone = set()

            def _mk_factor(f):
                def go():
                    emit_factor(f, 1)
                    davail.add(f)
                    for i, uv in enumerate(uv_order):
                        u, v = uv // 3 - 1, uv % 3 - 1
                        fy = {-1: "by", 0: "y0", 1: "ay"}[u]
                        fx = {-1: "bx", 0: "x0", 1: "ax"}[v]
                        if (uv in ddone or fy not in davail
                                or fx not in davail):
                            continue
                        ddone.add(uv)
                        emit_product(uv, 1, i)
                return go

            for f in ford:
                deferred.append(_mk_factor(f))

            # ---------------- phase 3: modulated accumulation ----------------
            # per-sweep wrow slots (slot j holds only this sweep's half)
            nslot = max(len(PE_SWEEP[0]), len(PE_SWEEP[1]), 1)
            wrows = cp.tile([2, nslot * SW], f16)
            out_sb = cp.tile([COUT, NPIX], f16)
            wexp_ps_warm = psw.tile([128, SW], f32, name="wexp_ps",
                                    tag="wexp_ps")
            for sweep in range(2):
                soff = SW * sweep
                srow = SROWS * sweep
                psum_out = ps_tile()
                for p, (ta, sa, tb, sb, split) in enumerate(PASSES):
                    if sweep == 0 and p >= 1 and p % 3 == 1 and deferred:
                        deferred.pop(0)()
                    nprt = 128 if tb is not None else 64
                    wexp = wp.tile([128, SW], f16)
                    ra = _wrow(ta)
                    if p in PE_SWEEP[sweep]:
                        # rows stored ascending (one pair-DMA per sweep
                        # into this sweep's slot); a flipped selector
                        # handles pairs whose partition order descends
                        j = PE_SWEEP[sweep].index(p)
                        ra, rb = _wrow(ta), _wrow(tb)
                        lo, hi = min(ra, rb), max(ra, rb)
                        nc.scalar.dma_start(
                            out=wrows[0:2, j * SW:(j + 1) * SW],
                            in_=bass.AP(wtab_h[sweep], lo * SW,
                                        [[(hi - lo) * SW, 2], [1, SW]]))
                        psel = sel if ra == lo else selflip
                        wexp_ps = psw.tile([128, SW], f32, name="wexp_ps",
                                           tag="wexp_ps")
                        for c in range(NCH):
                            nc.tensor.matmul(
                                wexp_ps[:, CH * c:CH * (c + 1)], psel,
                                wrows[0:2, j * SW + CH * c:
                                      j * SW + CH * (c + 1)],
                                start=True, stop=True)
                        nc.scalar.copy(out=wexp[:], in_=wexp_ps[:])
                    else:
                        gq = nc.sync
                        if tb is not None and _wrow(tb) > ra:
                            # one DMA broadcasts both rows (ascending pair)
                            rb = _wrow(tb)
                            gq.dma_start(
                                out=wexp[:, :],
                                in_=bass.AP(wtab_h[sweep], ra * SW,
                                            [[(rb - ra) * SW, 2],
                                             [0, 64], [1, SW]]))
                        else:
                            gq.dma_start(
                                out=wexp[0:64, :],
                                in_=bass.AP(wtab_h[sweep], ra * SW,
                                            [[0, 64], [1, SW]]))
                            if tb is not None:
                                gq.dma_start(
                                    out=wexp[64:128, :],
                                    in_=bass.AP(wtab_h[sweep],
                                                _wrow(tb) * SW,
                                                [[0, 64], [1, SW]]))
                    mod = mp.tile([128, SW], f16)

                    # modulate split by pixel rows across DVE (rows 0..DR2)
                    # and Pool (rest); Pool tensor_mul runs at 0.42 eff.
                    # Alternate 26/25 to equalize engine TOTALS (DVE also
                    # carries tents/evac work).
                    dr = DR2
                    DSPL = dr * W

                    def ivw(base, npart, poff, r0, r1):
                        return bass.AP(
                            xt.tensor,
                            xt.offset + poff * xps + base + (srow + r0) * HP,
                            [[xps, npart], [HP, r1 - r0], [1, W]])

                    def stt(lo, hi, base, poff):
                        nc.vector.tensor_mul(
                            mod[lo:hi, 0:DSPL],
                            ivw(base, hi - lo, poff, 0, dr),
                            wexp[lo:hi, 0:DSPL])
                        nc.gpsimd.tensor_mul(
                            mod[lo:hi, DSPL:SW],
                            ivw(base, hi - lo, poff, dr, SROWS),
                            wexp[lo:hi, DSPL:SW])

                    base1 = (sa[0] + 2) * HP + sa[1] + 2
                    if not split:
                        stt(0, 128, base1, 0)
                    elif tb is None:
                        stt(0, 64, base1, 0)
                    else:
                        base2 = (sb[0] + 1) * HP + sb[1] + 2
                        stt(0, 64, base1, 0)
                        stt(64, 128, base2, 64)
                    # in the final pass, stop chunks 2,3 first so the
                    # tail store of cols 2CH:SW can launch earlier
                    corder = (2, 3, 0, 1) if p == NP_ - 1 else range(NCH)
                    for c in corder:
                        nc.tensor.matmul(
                            psum_out[:, CH * c:CH * (c + 1)],
                            cst[0:nprt, 64 * p:64 * (p + 1)],
                            mod[0:nprt, CH * c:CH * (c + 1)],
                            start=(p == 0),
                            stop=(p == NP_ - 1),
                        )
                    if sweep == 0 and p < 8:
                        # dep-free filler keeps the p-state ramp alive while
                        # the broadcast/modulate pipeline fills
                        for w in range(2):
                            nc.tensor.matmul(
                                wexp_ps_warm[0:2 * K2, 0:CH],
                                scratch[:, 0:2 * K2], scratch[:],
                                start=True, stop=True)

                # ---------------- phase 4: bias + store ----------------
                # one wide evac per store half, on separate engines, so
                # both output stores launch as soon as possible
                # (Pool cannot read PSUM on hardware: ACT/DVE only)
                nc.vector.tensor_scalar(
                    out_sb[:, soff + 2 * CH:soff + SW],
                    psum_out[:, 2 * CH:SW], db_sb, 0.0,
                    OP.add, OP.bypass)
                nc.sync.dma_start(out=y_d[:, soff + 2 * CH:soff + SW],
                                  in_=out_sb[:, soff + 2 * CH:soff + SW])
                nc.scalar.activation(out_sb[:, soff:soff + 2 * CH],
                                     psum_out[:, 0:2 * CH],
                                     AF.Identity, bias=db_sb, scale=1.0)
                nc.sync.dma_start(out=y_d[:, soff:soff + 2 * CH],
                                  in_=out_sb[:, soff:soff + 2 * CH])

    nc.compile()
    return nc


_NC = None


def _get_nc():
    global _NC
    if _NC is None:
        _NC = build_nc()
    return _NC


def _prep_shared(p_w, p_b, d_w, d_b):
    # permute offset channels to [dy_0..dy_8, dx_0..dx_8] (see phase 2)
    perm = list(range(0, 18, 2)) + list(range(1, 18, 2))
    p_w = p_w[perm]
    p_b = p_b[perm]
    pwt = np.zeros((128, 108), np.float16)
    for g in range(6):
        kw = g % 3
        if g < 3:
            pwt[0:64, 18 * g:18 * g + 18] = p_w[:, :, 0, kw].T
            pwt[64:128, 18 * g:18 * g + 18] = p_w[:, :, 1, kw].T
        else:
            pwt[64:128, 18 * g:18 * g + 18] = p_w[:, :, 2, kw].T
    cst = np.zeros((128, NCT), np.float16)
    cst[0, NCC:NCC + 64] = 1.0
    cst[1, NCC + 64:NCC + 128] = 1.0
    cst[1, NCC + 128:NCC + 192] = 1.0
    cst[0, NCC + 192:NCC + 256] = 1.0
    for p, (ta, sa, tb, sb, split) in enumerate(PASSES):
        c0 = 64 * p
        k = ta[0]
        cst[0:64, c0:c0 + 64] = d_w[:, :, k // 3, k % 3].T
        if tb is not None:
            k = tb[0]
            cst[64:128, c0:c0 + 64] = d_w[:, :, k // 3, k % 3].T
    bia = np.zeros((128, 2), np.float32)
    bia[0:2 * K2, 0] = p_b
    bia[0:COUT, 1] = d_b
    return pwt, cst, bia


def _prep_xpad(xb):
    """[128, FP] fp16: rows 0-63 x at (2,2); rows 64-127 x at (1,2)."""
    xp = np.zeros((128, HP, HP), np.float16)
    xp[0:64, 2:2 + H, 2:2 + W] = xb
    xp[64:128, 1:1 + H, 2:2 + W] = xb
    return xp.reshape(128, FP)


def kernel(x, p_w, p_b, d_w, d_b):
    x = np.asarray(x, np.float32)
    p_w = np.asarray(p_w, np.float32)
    p_b = np.asarray(p_b, np.float32)
    d_w = np.asarray(d_w, np.float32)
    d_b = np.asarray(d_b, np.float32)

    pwt, cst, bia = _prep_shared(p_w, p_b, d_w, d_b)
    in_maps = [{"xp": _prep_xpad(x[b]), "pwt": pwt, "cst": cst, "bia": bia}
               for b in range(B)]
    nc = _get_nc()
    res = run_bass_kernel_spmd(nc, in_maps, core_ids=list(range(B)))
    out = np.stack([res.results[b]["y"].reshape(COUT, H, W) for b in range(B)])
    return out.astype(np.float32)
